# revision 1
# baseline (speedup 1.0000x reference)
"""Causal self-attention (RMSNorm QK, key-gated ALiBi bias) on 8 TRN2 cores.

Sharding: data-parallel over batch (2) x tensor-parallel over heads (4 groups
of 4 heads) = 8 cores. Each core computes a partial c_proj output for its
batch; the host sums the 4 head-group partials per batch.

Device kernel (identical SPMD program, per-core data):
  P2  QKV projections (f32r matmuls, x^T resident in SBUF)
      q^T/k^T per 2-head pack -> per-head augmented tiles
      [66, T]: rows 0-63 rms-normalized q (or k), rows 64-65 bias terms.
  P3  key-gate: glog = u_n . k_norm / sqrt(D); gate = softplus via exp/ln;
      bias rows a[j] = j*w[j], -w[j] with w = softplus(omega)*slope*gate.
  P4  scores^T[j, i] = k_aug . q_aug (bias folded into the contraction),
      causal stair mask added pre-exp on diagonal blocks, exp on ACT,
      PV matmul with a ones-column in v giving softmax denominators free.
  P5  y normalized via exp(-ln(denom)) broadcast, packed 2 heads/tile,
      c_proj matmul, partial output to DRAM.
"""

import sys

if "/opt/trn_rl_repo" not in sys.path:
    sys.path.insert(0, "/opt/trn_rl_repo")

import math

import numpy as np

B, T, C = 2, 2048, 1024
H, D = 16, 64
HLOC = 4           # heads per core
HD = HLOC * D      # 256
NCH = 512          # i-chunk width
NT = T // NCH      # 4 i-chunks
JT = T // 128      # 16 j-tiles
KC = C // 128      # 8 contraction chunks
EPS_RMS = 1e-5
U_L2_EPS = 1e-6
NEG_BIG = -1.0e30

_cache = {}


def _get_alibi_slopes(n_heads):
    def pow2(n):
        start = 2 ** (-(2 ** (-(math.log2(n) - 3))))
        return [start * start**i for i in range(n)]

    if math.log2(n_heads).is_integer():
        return pow2(n_heads)
    c = 2 ** math.floor(math.log2(n_heads))
    s = pow2(c)
    extra = _get_alibi_slopes(2 * c)
    return s + extra[0::2][: n_heads - c]


def _build_program():
    import concourse.bass as bass
    import concourse.mybir as mybir
    import concourse.tile as tile
    from concourse.alu_op_type import AluOpType
    from concourse.vector_clock import ScopedClock

    F32 = mybir.dt.float32
    F32R = mybir.dt.float32r
    BF16 = mybir.dt.bfloat16
    AF = mybir.ActivationFunctionType
    MUL = AluOpType.mult
    ADD = AluOpType.add
    SUB = AluOpType.subtract

    class PatchedTileContext(tile.TileContext):
        """Tail drain split into nops carrying <=2 sem waits each (this
        walrus build rejects CTRL instructions with more)."""

        def _drain_and_barrier(self, tick_clock, wait_clock):
            nc = self.nc
            probe = nc.sync.nop(nofuse=True)
            wait_clock.add_sem_waits(
                probe.ins, ScopedClock({None: tick_clock.global_clock})
            )
            si = probe.ins.sync_info
            waits = list(si.on_wait or []) if si is not None else []
            if len(waits) > 2:
                si.on_wait = waits[:2]
                rest = waits[2:]
                for i in range(0, len(rest), 2):
                    extra = nc.sync.nop(nofuse=True)
                    esi = extra.ins.sync_info
                    chunk = rest[i : i + 2]
                    if esi is None:
                        extra.ins.sync_info = mybir.SyncInfo(
                            on_wait=chunk, on_update=[]
                        )
                    else:
                        esi.on_wait = (esi.on_wait or []) + chunk
            nc.sync.drain()
            nc.all_engine_barrier()
            assert self.sems is not None
            popped = nc._tile_sem_poison_stack.pop()
            assert popped is self._sem_poison
            nc.clear_and_free_semaphores(list(self.sems.allocated().values()))
            nc.all_engine_barrier()

    def split_excess_waits(nc, max_waits=1):
        for f in nc.m.functions:
            for blk in f.blocks:
                new_insts = []
                for inst in blk.instructions:
                    si = inst.sync_info
                    if si is not None and si.on_wait and len(si.on_wait) > max_waits:
                        waits = list(si.on_wait)
                        si.on_wait = waits[-max_waits:]
                        rest = waits[:-max_waits]
                        for i in range(0, len(rest), max_waits):
                            nop = mybir.InstNoOp(
                                name=f"I-waitsplit-{nc.next_id()}",
                                ins=[],
                                outs=[],
                                engine=inst.engine,
                                sync_info=mybir.SyncInfo(
                                    on_wait=rest[i : i + max_waits], on_update=[]
                                ),
                            )
                            nc.register_instruction(nop)
                            new_insts.append(nop)
                    new_insts.append(inst)
                blk.instructions = new_insts

    nc = bass.Bass(trn_type="TRN2", num_devices=8, debug=False)

    # ---- DRAM I/O (per-core shards supplied by the host) ----
    d_xT = nc.dram_tensor("xT", [C, T], F32, kind="ExternalInput")
    d_wq = nc.dram_tensor("wq", [C, HD], F32, kind="ExternalInput")
    d_wk = nc.dram_tensor("wk", [C, HD], F32, kind="ExternalInput")
    d_wv = nc.dram_tensor("wv", [C, HD], F32, kind="ExternalInput")
    d_wproj = nc.dram_tensor("wproj", [HD, C], F32, kind="ExternalInput")
    d_ucol = nc.dram_tensor("ucol", [D, HLOC], F32, kind="ExternalInput")
    d_omg = nc.dram_tensor("omg", [HLOC, 1], F32, kind="ExternalInput")
    d_negomg = nc.dram_tensor("negomg", [HLOC, 1], F32, kind="ExternalInput")
    d_iota4 = nc.dram_tensor("iota4", [HLOC, T], F32, kind="ExternalInput")
    d_oh16 = nc.dram_tensor("oh16", [1, 16], F32, kind="ExternalInput")
    d_iota = nc.dram_tensor("iota", [1, T], F32, kind="ExternalInput")
    d_ones = nc.dram_tensor("ones_row", [1, T], F32, kind="ExternalInput")
    d_ones4 = nc.dram_tensor("ones4", [128, HLOC], BF16, kind="ExternalInput")
    d_stair = nc.dram_tensor("stair", [128, 128], F32, kind="ExternalInput")
    d_ssqw = nc.dram_tensor("ssqw", [128, 2], F32, kind="ExternalInput")
    d_qw8 = nc.dram_tensor("qw8", [128, 1], F32, kind="ExternalInput")
    d_kw = nc.dram_tensor("kw", [128, 1], F32, kind="ExternalInput")
    d_out = nc.dram_tensor("out", [T, C], F32, kind="ExternalOutput")

    with PatchedTileContext(nc) as tc:
        from contextlib import ExitStack

        with ExitStack() as top:
            persist = top.enter_context(tc.tile_pool(name="persist", bufs=1))

            # ---- persistent SBUF tensors ----
            q_aug = [persist.tile([68, T], F32R, tag=f"qaug{h}", name=f"qaug{h}") for h in range(HLOC)]
            k_aug = [persist.tile([68, T], F32R, tag=f"kaug{h}", name=f"kaug{h}") for h in range(HLOC)]
            v_sb = [
                persist.tile([128, HLOC * 65], BF16, tag=f"vsb{t}", name=f"vsb{t}") for t in range(JT)
            ]
            stair = persist.tile([128, 128], F32, tag="stair", name="stair")
            nc.sync.dma_start(stair[:], d_stair[:])
            ssqw = persist.tile([128, 2], F32R, tag="ssqw", name="ssqw")
            nc.sync.dma_start(ssqw[:], d_ssqw[:].bitcast(F32R))
            ucol = persist.tile([D, HLOC], F32R, tag="ucol", name="ucol")
            nc.sync.dma_start(ucol[:], d_ucol[:].bitcast(F32R))
            omg = persist.tile([HLOC, 1], F32, tag="omg", name="omg")
            nc.sync.dma_start(omg[:], d_omg[:])
            negomg = persist.tile([HLOC, 1], F32, tag="negomg", name="negomg")
            oh16 = persist.tile([1, 16], F32R, tag="oh16", name="oh16")
            nc.sync.dma_start(oh16[:], d_oh16[:].bitcast(F32R))
            nc.sync.dma_start(negomg[:], d_negomg[:])
            qw8 = persist.tile([128, 1], F32, tag="qw8", name="qw8")
            nc.sync.dma_start(qw8[:], d_qw8[:])
            kw = persist.tile([128, 1], F32, tag="kw", name="kw")
            nc.sync.dma_start(kw[:], d_kw[:])
            eps_col = persist.tile([128, 1], F32, tag="eps", name="eps")
            nc.vector.memset(eps_col[:], EPS_RMS)
            neghalf_col = persist.tile([128, 1], F32, tag="neghalf", name="neghalf")
            nc.vector.memset(neghalf_col[:], -0.5)
            neg1_col = persist.tile([128, 1], F32, tag="neg1", name="neg1")
            nc.vector.memset(neg1_col[:], -1.0)
            one_col = persist.tile([128, 1], F32, tag="onec", name="onec")
            nc.vector.memset(one_col[:], 1.0)

            # aug fixed rows: q rows 64 (ones) / 65 (iota)
            for h in range(HLOC):
                nc.sync.dma_start(q_aug[h][64:65, :], d_ones[:].bitcast(F32R))
                nc.sync.dma_start(q_aug[h][65:66, :], d_ones[:].bitcast(F32R))
                nc.sync.dma_start(q_aug[h][66:67, :], d_iota[:].bitcast(F32R))
                nc.sync.dma_start(q_aug[h][67:68, :], d_iota[:].bitcast(F32R))

            # v ones columns
            for t in range(JT):
                dst = v_sb[t][:].rearrange("p (h d) -> p h d", h=HLOC)[:, :, 64:65]
                nc.sync.dma_start(dst, d_ones4[:].rearrange("p (h o) -> p h o", o=1))

            # ================= P2: QKV projections =================
            with ExitStack() as p2:
                xpool = p2.enter_context(tc.tile_pool(name="xT", bufs=1))
                xT = []
                for cc in range(KC):
                    t = xpool.tile([128, T], F32R, tag=f"xT{cc}", name=f"xT{cc}")
                    nc.sync.dma_start(
                        t[:], d_xT[128 * cc : 128 * cc + 128, :].bitcast(F32R)
                    )
                    xT.append(t)

                wpool = p2.enter_context(tc.tile_pool(name="w", bufs=1))
                qk_ps = p2.enter_context(
                    tc.tile_pool(name="qkps", bufs=2, space="PSUM")
                )
                ssq_ps = p2.enter_context(
                    tc.tile_pool(name="ssqps", bufs=2, space="PSUM")
                )
                sq_pool = p2.enter_context(tc.tile_pool(name="qsq", bufs=2))
                rsq_pool = p2.enter_context(tc.tile_pool(name="rsq", bufs=3))
                rep_pool = p2.enter_context(tc.tile_pool(name="rep", bufs=4))
                dram = p2.enter_context(
                    tc.tile_pool(name="dram", bufs=16, space="DRAM")
                )

                w_sb = {}
                for name, dten in (("q", d_wq), ("k", d_wk)):
                    for cc in range(KC):
                        wt = wpool.tile([128, HD], F32R, tag=f"w{name}{cc}", name=f"w{name}{cc}")
                        nc.sync.dma_start(
                            wt[:], dten[128 * cc : 128 * cc + 128, :].bitcast(F32R)
                        )
                        w_sb[(name, cc)] = wt

                for p in range(2):  # head pack
                    for n in range(NT):  # T chunk
                        sl = slice(NCH * n, NCH * n + NCH)
                        for name, wcol, proj_w in (
                            ("q", qw8, "q"),
                            ("k", kw, "k"),
                        ):
                            ps = qk_ps.tile([128, NCH], F32, tag="qk", name="qk")
                            for cc in range(KC):
                                nc.tensor.matmul(
                                    ps[:],
                                    w_sb[(name, cc)][:, 128 * p : 128 * p + 128],
                                    xT[cc][:, sl],
                                    start=(cc == 0),
                                    stop=(cc == KC - 1),
                                )
                            # sum of squares per head slot -> mean
                            qsq = sq_pool.tile([128, NCH], F32R, tag="qsq", name="qsq")
                            nc.scalar.activation(qsq[:], ps[:], AF.Square)
                            ssq = ssq_ps.tile([2, NCH], F32, tag="ssq", name="ssq")
                            nc.tensor.matmul(
                                ssq[:], ssqw[:], qsq[:], start=True, stop=True
                            )
                            # rsqrt(mean + eps) = exp(-0.5 ln(mean + eps))
                            rsq = rsq_pool.tile([2, NCH], F32, tag="rsq", name="rsq")
                            nc.scalar.activation(
                                rsq[:], ssq[:], AF.Ln, bias=eps_col[0:2, :]
                            )
                            nc.scalar.activation(
                                rsq[:], rsq[:], AF.Exp, scale=neghalf_col[0:2, :]
                            )
                            for s in range(2):  # head slot in pack
                                h = 2 * p + s
                                drow = dram.tile([1, NCH], F32, tag="drsq", name="drsq")
                                nc.sync.dma_start(drow[:], rsq[s : s + 1, :])
                                rep = rep_pool.tile([64, NCH], F32, tag="rep", name="rep")
                                nc.sync.dma_start(
                                    rep[:], drow[0:1, :].partition_broadcast(64)
                                )
                                aug = q_aug[h] if name == "q" else k_aug[h]
                                nc.vector.scalar_tensor_tensor(
                                    aug[0:64, sl],
                                    ps[64 * s : 64 * s + 64, :],
                                    wcol[0:64, :],
                                    rep[:],
                                    MUL,
                                    MUL,
                                )

                # free q/k weights, load v weights
                wv_sb = []
                for cc in range(KC):
                    wt = wpool.tile([128, HD], F32R, tag=f"wq{cc}", name=f"wv{cc}")
                    nc.sync.dma_start(
                        wt[:], d_wv[128 * cc : 128 * cc + 128, :].bitcast(F32R)
                    )
                    wv_sb.append(wt)
                v_ps_pool = p2.enter_context(
                    tc.tile_pool(name="vps", bufs=2, space="PSUM")
                )
                for t in range(JT):
                    vps = v_ps_pool.tile([128, HD], F32, tag="vps", name="vps")
                    for cc in range(KC):
                        nc.tensor.matmul(
                            vps[:],
                            xT[cc][:, 128 * t : 128 * t + 128],
                            wv_sb[cc][:],
                            start=(cc == 0),
                            stop=(cc == KC - 1),
                        )
                    dst = v_sb[t][:].rearrange("p (h d) -> p h d", h=HLOC)[:, :, 0:64]
                    nc.vector.tensor_copy(
                        dst, vps[:].rearrange("p (h d) -> p h d", h=HLOC)
                    )

                # ================= P3: key gate / bias rows =================
                glog_ps = p2.enter_context(
                    tc.tile_pool(name="glps", bufs=1, space="PSUM")
                )
                gate4_ps = p2.enter_context(
                    tc.tile_pool(name="g4ps", bufs=1, space="PSUM")
                )
                grow_pool = p2.enter_context(tc.tile_pool(name="grow", bufs=2))
                iot_pool = p2.enter_context(tc.tile_pool(name="iot", bufs=2))
                for n in range(NT):
                    sl = slice(NCH * n, NCH * n + NCH)
                    g4 = gate4_ps.tile([HLOC, NCH], F32, tag="g4", name="g4")
                    for h in range(HLOC):
                        gl = glog_ps.tile([1, NCH], F32, tag="glog", name="glog")
                        nc.tensor.matmul(
                            gl[:],
                            ucol[:, h : h + 1],
                            k_aug[h][0:64, sl],
                            start=True,
                            stop=True,
                        )
                        gsc = grow_pool.tile([1, NCH], F32, tag="gsc", name="gsc")
                        nc.scalar.activation(gsc[:], gl[:], AF.Exp)
                        gate = grow_pool.tile([1, NCH], F32R, tag="gate", name="gate")
                        nc.scalar.activation(
                            gate[:], gsc[:], AF.Ln, bias=one_col[0:1, :]
                        )
                        nc.tensor.matmul(
                            g4[:],
                            oh16[0:1, 4 * h : 4 * h + 4],
                            gate[:],
                            start=(h == 0),
                            stop=(h == HLOC - 1),
                            skip_group_check=True,
                        )
                    iots = iot_pool.tile([HLOC, NCH], F32, tag="iot", name="iot")
                    nc.sync.dma_start(iots[:], d_iota4[:, sl])
                    a4f = grow_pool.tile([HLOC, NCH], F32, tag="a4f", name="a4f")
                    nc.vector.scalar_tensor_tensor(
                        a4f[:], g4[:], omg[:], iots[:], MUL, MUL
                    )
                    w4f = grow_pool.tile([HLOC, NCH], F32, tag="w4f", name="w4f")
                    nc.vector.tensor_scalar_mul(w4f[:], g4[:], negomg[:])
                    a_hi = grow_pool.tile([HLOC, NCH], F32R, tag="a_hi", name="a_hi")
                    nc.vector.tensor_copy(a_hi[:], a4f[:])
                    a_lo = grow_pool.tile([HLOC, NCH], F32R, tag="a_lo", name="a_lo")
                    nc.vector.scalar_tensor_tensor(
                        a_lo[:], a4f[:], 1.0, a_hi[:].bitcast(F32), MUL, SUB
                    )
                    w_hi = grow_pool.tile([HLOC, NCH], F32R, tag="w_hi", name="w_hi")
                    nc.vector.tensor_copy(w_hi[:], w4f[:])
                    w_lo = grow_pool.tile([HLOC, NCH], F32R, tag="w_lo", name="w_lo")
                    nc.vector.scalar_tensor_tensor(
                        w_lo[:], w4f[:], 1.0, w_hi[:].bitcast(F32), MUL, SUB
                    )
                    for h in range(HLOC):
                        nc.sync.dma_start(k_aug[h][64:65, sl], a_hi[h : h + 1, :])
                        nc.sync.dma_start(k_aug[h][65:66, sl], a_lo[h : h + 1, :])
                        nc.sync.dma_start(k_aug[h][66:67, sl], w_hi[h : h + 1, :])
                        nc.sync.dma_start(k_aug[h][67:68, sl], w_lo[h : h + 1, :])

            # ================= P4 + P5: attention & projection =============
            with ExitStack() as p4:
                s_ps_pool = p4.enter_context(
                    tc.tile_pool(name="sps", bufs=3, space="PSUM")
                )
                y_ps_pool = p4.enter_context(
                    tc.tile_pool(name="yps", bufs=2, space="PSUM")
                )
                o_ps_pool = p4.enter_context(
                    tc.tile_pool(name="ops", bufs=2, space="PSUM")
                )
                p_pool = p4.enter_context(tc.tile_pool(name="p", bufs=6))
                den_pool = p4.enter_context(tc.tile_pool(name="den", bufs=3))
                rep2_pool = p4.enter_context(tc.tile_pool(name="rep2", bufs=4))
                out_pool = p4.enter_context(tc.tile_pool(name="osb", bufs=3))
                dram2 = p4.enter_context(
                    tc.tile_pool(name="dram2", bufs=4, space="DRAM")
                )
                wp_pool = p4.enter_context(tc.tile_pool(name="wp", bufs=1))
                y_pack = [
                    wp_pool.tile([128, T], F32R, tag=f"ypk{p}", name=f"ypk{p}")
                    for p in range(2)
                ]
                wproj_sb = []
                for p in range(2):
                    wt = wp_pool.tile([128, C], F32R, tag=f"wp{p}", name=f"wp{p}")
                    nc.sync.dma_start(
                        wt[:], d_wproj[128 * p : 128 * p + 128, :].bitcast(F32R)
                    )
                    wproj_sb.append(wt)

                for ci in range(NT):
                    isl = slice(NCH * ci, NCH * ci + NCH)
                    for h in range(HLOC):
                        p_pk, s_slot = divmod(h, 2)
                        yps = y_ps_pool.tile([65, NCH], F32, tag="yps", name="yps")
                        last_tj = 4 * ci + 3
                        for tj in range(4 * ci + 4):
                            sps = s_ps_pool.tile([128, NCH], F32, tag="sps", name="sps")
                            nc.tensor.matmul(
                                sps[:],
                                k_aug[h][:, 128 * tj : 128 * tj + 128],
                                q_aug[h][:, isl],
                                start=True,
                                stop=True,
                            )
                            r = tj - 4 * ci
                            if r < 0:
                                off = 0
                            else:
                                off = 128 * r
                                nc.vector.tensor_tensor(
                                    sps[:, off : off + 128],
                                    sps[:, off : off + 128],
                                    stair[:],
                                    ADD,
                                )
                            psb = p_pool.tile([128, NCH], BF16, tag="p", name="p")
                            nc.scalar.activation(
                                psb[:, off:NCH], sps[:, off:NCH], AF.Exp
                            )
                            nc.tensor.matmul(
                                yps[:, off:NCH],
                                v_sb[tj][:, 65 * h : 65 * h + 65],
                                psb[:, off:NCH],
                                start=(tj == 0),
                                stop=(tj == last_tj),
                                skip_group_check=True,
                            )
                        # normalize: 1/denom = exp(-ln(denom))
                        d1 = den_pool.tile([65, NCH], F32, tag="d1", name="d1")
                        nc.scalar.activation(d1[64:65, :], yps[64:65, :], AF.Ln)
                        nc.scalar.activation(
                            d1[64:65, :],
                            d1[64:65, :],
                            AF.Exp,
                            scale=neg1_col[64:65, :],
                        )
                        drow = dram2.tile([1, NCH], F32, tag="dden", name="dden")
                        nc.sync.dma_start(drow[:], d1[64:65, :])
                        rrep = rep2_pool.tile([64, NCH], F32, tag="rrep", name="rrep")
                        nc.sync.dma_start(
                            rrep[:], drow[0:1, :].partition_broadcast(64)
                        )
                        nc.vector.tensor_tensor(
                            y_pack[p_pk][64 * s_slot : 64 * s_slot + 64, isl],
                            yps[0:64, :],
                            rrep[:],
                            MUL,
                        )
                    # P5: projection for the 4 t-tiles covered by this chunk
                    for tt in range(4 * ci, 4 * ci + 4):
                        for cn in range(2):
                            osl = slice(512 * cn, 512 * cn + 512)
                            ops = o_ps_pool.tile([128, 512], F32, tag="ops", name="ops")
                            for p in range(2):
                                nc.tensor.matmul(
                                    ops[:],
                                    y_pack[p][:, 128 * tt : 128 * tt + 128],
                                    wproj_sb[p][:, osl],
                                    start=(p == 0),
                                    stop=(p == 1),
                                )
                            osb = out_pool.tile([128, 512], F32, tag="osb", name="osb")
                            nc.vector.tensor_copy(osb[:], ops[:])
                            nc.sync.dma_start(
                                d_out[128 * tt : 128 * tt + 128, osl], osb[:]
                            )

    split_excess_waits(nc, max_waits=1)
    return nc


def _host_shards(inputs):
    x = np.asarray(inputs["x"], np.float32)
    Wq = np.asarray(inputs["Wq"], np.float32)
    Wk = np.asarray(inputs["Wk"], np.float32)
    Wv = np.asarray(inputs["Wv"], np.float32)
    Wproj = np.asarray(inputs["Wproj"], np.float32)
    q_rms_w = np.asarray(inputs["q_rms_w"], np.float32)
    k_rms_w = np.asarray(inputs["k_rms_w"], np.float32)
    omega = np.asarray(inputs["omega"], np.float32)
    u = np.asarray(inputs["u"], np.float32)

    import ml_dtypes

    slopes = np.asarray(_get_alibi_slopes(H), np.float32)
    omega_eff = np.log1p(np.exp(omega)) * slopes  # softplus(omega) * slopes
    u_n = u / np.maximum(
        np.linalg.norm(u, axis=-1, keepdims=True), U_L2_EPS
    )
    sqrt_d = math.sqrt(D)

    iota = np.arange(T, dtype=np.float32)[None, :]
    ones_row = np.ones((1, T), np.float32)
    ones4 = np.ones((128, HLOC), np.float32).astype(ml_dtypes.bfloat16)
    jj = np.arange(128, dtype=np.float32)
    stair = np.where(jj[None, :] >= jj[:, None], 0.0, NEG_BIG).astype(np.float32)
    ssqw = np.zeros((128, 2), np.float32)
    ssqw[0:64, 0] = 1.0 / D
    ssqw[64:128, 1] = 1.0 / D
    qw8 = np.tile(q_rms_w / (8.0), 2)[:, None].astype(np.float32)
    kw = np.tile(k_rms_w, 2)[:, None].astype(np.float32)

    in_maps = []
    for core in range(8):
        b, g = divmod(core, HLOC)
        hs = slice(HLOC * g, HLOC * g + HLOC)
        cs = slice(HD * g, HD * g + HD)
        in_maps.append(
            {
                "xT": np.ascontiguousarray(x[b].T),
                "wq": np.ascontiguousarray(Wq[:, cs]),
                "wk": np.ascontiguousarray(Wk[:, cs]),
                "wv": np.ascontiguousarray(Wv[:, cs]),
                "wproj": np.ascontiguousarray(Wproj[cs, :]),
                "ucol": np.ascontiguousarray(u_n[hs].T / sqrt_d),
                "omg": np.ascontiguousarray(omega_eff[hs][:, None]),
                "negomg": np.ascontiguousarray(-omega_eff[hs][:, None]),
                "iota4": np.tile(iota, (HLOC, 1)),
                "oh16": np.eye(HLOC, dtype=np.float32).reshape(1, 16),
                "iota": iota,
                "ones_row": ones_row,
                "ones4": ones4,
                "stair": stair,
                "ssqw": ssqw,
                "qw8": qw8,
                "kw": kw,
            }
        )
    return in_maps


def kernel(**inputs):
    from concourse.bass_utils import run_bass_kernel_spmd

    if "nc" not in _cache:
        _cache["nc"] = _build_program()
    nc = _cache["nc"]

    in_maps = _host_shards(inputs)
    res = run_bass_kernel_spmd(nc, in_maps, core_ids=list(range(8)))
    out = np.zeros((B, T, C), np.float32)
    for core in range(8):
        b = core // HLOC
        out[b] += res.results[core]["out"]
    return out



# revision 11
# speedup vs baseline: 1.4751x; 1.4751x over previous
"""Causal self-attention (RMSNorm QK, key-gated ALiBi bias) on 8 TRN2 cores.

Sharding: data-parallel over batch (2) x tensor-parallel over heads (4 groups
of 4 heads) = 8 cores. Each core computes a partial c_proj output for its
batch; the host sums the 4 head-group partials per batch.

Device kernel (identical SPMD program, per-core data):
  P2  QKV projections (bf16 matmuls, xT bf16 resident in SBUF).
      Per (pack p, chunk n): q/k PSUM -> Square(ACT) -> sum-sq matmul into a
      partition-spread [97,512] PSUM ({0,32,64,96} rows) -> ln/exp rsqrt ->
      matmul-broadcast (sel x rsq) to [128,512] PSUM -> DVE STT writes
      normalized q/k into per-head augmented tiles [68,T] f32r.
      Software-pipelined so the in-order PE never waits on ACT.
  P3  key-gate: per-head gate logits matmul into partition-spread PSUM rows,
      one exp per chunk, one wide ln, wide DVE ops build a=j*w / w rows with
      f32r hi/lo splits, DMA'd into k_aug rows 64..67.
  P4  scores^T = k_aug . q_aug in [128,1024] PSUM pairs (bias folded into the
      contraction), one exp per pair (ACT), causal wedge masked via bf16 MIN
      on the diagonal blocks (DVE 4x), PV matmul with a ones-column in v
      giving softmax denominators free.
  P5  y normalized via DVE reciprocal + DRAM-roundtrip broadcast, packed
      2 heads/tile bf16, c_proj matmul interleaved into the next chunk's
      attention, partial output to DRAM.
"""

import sys

if "/opt/trn_rl_repo" not in sys.path:
    sys.path.insert(0, "/opt/trn_rl_repo")

import math

import numpy as np

B, T, C = 2, 2048, 1024
H, D = 16, 64
HLOC = 4           # heads per core
HD = HLOC * D      # 256
NCH = 512          # i-chunk width
NT = T // NCH      # 4 i-chunks
JT = T // 128      # 16 j-tiles
KC = C // 128      # 8 contraction chunks
EPS_RMS = 1e-5
U_L2_EPS = 1e-6
BIGF = 3.0e38      # bf16-representable "+inf" for the MIN mask

_cache = {}


def _get_alibi_slopes(n_heads):
    def pow2(n):
        start = 2 ** (-(2 ** (-(math.log2(n) - 3))))
        return [start * start**i for i in range(n)]

    if math.log2(n_heads).is_integer():
        return pow2(n_heads)
    c = 2 ** math.floor(math.log2(n_heads))
    s = pow2(c)
    extra = _get_alibi_slopes(2 * c)
    return s + extra[0::2][: n_heads - c]


def _build_program():
    import concourse.bass as bass
    import concourse.mybir as mybir
    import concourse.tile as tile
    from concourse.alu_op_type import AluOpType
    from concourse.vector_clock import ScopedClock

    F32 = mybir.dt.float32
    F32R = mybir.dt.float32r
    BF16 = mybir.dt.bfloat16
    AF = mybir.ActivationFunctionType
    MUL = AluOpType.mult
    ADD = AluOpType.add
    SUB = AluOpType.subtract
    MIN = AluOpType.min

    class PatchedTileContext(tile.TileContext):
        """Tail drain split into nops carrying <=2 sem waits each (this
        walrus build rejects CTRL instructions with more)."""

        def _drain_and_barrier(self, tick_clock, wait_clock):
            nc = self.nc
            probe = nc.sync.nop(nofuse=True)
            wait_clock.add_sem_waits(
                probe.ins, ScopedClock({None: tick_clock.global_clock})
            )
            si = probe.ins.sync_info
            waits = list(si.on_wait or []) if si is not None else []
            if len(waits) > 2:
                si.on_wait = waits[:2]
                rest = waits[2:]
                for i in range(0, len(rest), 2):
                    extra = nc.sync.nop(nofuse=True)
                    esi = extra.ins.sync_info
                    chunk = rest[i : i + 2]
                    if esi is None:
                        extra.ins.sync_info = mybir.SyncInfo(
                            on_wait=chunk, on_update=[]
                        )
                    else:
                        esi.on_wait = (esi.on_wait or []) + chunk
            nc.sync.drain()
            nc.all_engine_barrier()
            assert self.sems is not None
            popped = nc._tile_sem_poison_stack.pop()
            assert popped is self._sem_poison
            nc.clear_and_free_semaphores(list(self.sems.allocated().values()))
            nc.all_engine_barrier()

    def split_excess_waits(nc, max_waits=1):
        for f in nc.m.functions:
            for blk in f.blocks:
                new_insts = []
                for inst in blk.instructions:
                    si = inst.sync_info
                    if si is not None and si.on_wait and len(si.on_wait) > max_waits:
                        waits = list(si.on_wait)
                        si.on_wait = waits[-max_waits:]
                        rest = waits[:-max_waits]
                        for i in range(0, len(rest), max_waits):
                            nop = mybir.InstNoOp(
                                name=f"I-waitsplit-{nc.next_id()}",
                                ins=[],
                                outs=[],
                                engine=inst.engine,
                                sync_info=mybir.SyncInfo(
                                    on_wait=rest[i : i + max_waits], on_update=[]
                                ),
                            )
                            nc.register_instruction(nop)
                            new_insts.append(nop)
                    new_insts.append(inst)
                blk.instructions = new_insts

    nc = bass.Bass(trn_type="TRN2", num_devices=8, debug=False)

    # ---- DRAM I/O (per-core shards supplied by the host) ----
    d_xT = nc.dram_tensor("xT", [C, T], BF16, kind="ExternalInput")
    d_wq = nc.dram_tensor("wq", [C, HD], BF16, kind="ExternalInput")
    d_wk = nc.dram_tensor("wk", [C, HD], BF16, kind="ExternalInput")
    d_wv = nc.dram_tensor("wv", [C, HD], BF16, kind="ExternalInput")
    d_wproj = nc.dram_tensor("wproj", [HD, C], BF16, kind="ExternalInput")
    d_ssqw_q = nc.dram_tensor("ssqw_q", [128, 97], F32, kind="ExternalInput")
    d_ssqw_k = nc.dram_tensor("ssqw_k", [128, 97], F32, kind="ExternalInput")
    d_sel_q = nc.dram_tensor("sel_q", [97, 128], F32, kind="ExternalInput")
    d_sel_k = nc.dram_tensor("sel_k", [97, 128], F32, kind="ExternalInput")
    d_ucol = nc.dram_tensor("ucol", [D, 512], F32, kind="ExternalInput")
    d_omg97 = nc.dram_tensor("omg97", [128, 1], F32, kind="ExternalInput")
    d_negomg97 = nc.dram_tensor("negomg97", [128, 1], F32, kind="ExternalInput")
    d_iota97 = nc.dram_tensor("iota97", [128, T], F32, kind="ExternalInput")
    d_iota = nc.dram_tensor("iota", [1, T], F32, kind="ExternalInput")
    d_ones = nc.dram_tensor("ones_row", [1, T], F32, kind="ExternalInput")
    d_ones4 = nc.dram_tensor("ones4", [128, HLOC], BF16, kind="ExternalInput")
    d_stair01 = nc.dram_tensor("stair01", [128, 128], BF16, kind="ExternalInput")
    d_qw8 = nc.dram_tensor("qw8", [128, 1], F32, kind="ExternalInput")
    d_kw = nc.dram_tensor("kw", [128, 1], F32, kind="ExternalInput")
    d_out = nc.dram_tensor("out", [T, C], F32, kind="ExternalOutput")

    with PatchedTileContext(nc) as tc:
        from contextlib import ExitStack

        with ExitStack() as top:
            persist = top.enter_context(tc.tile_pool(name="persist", bufs=1))

            # ---- persistent SBUF tensors ----
            q_aug = [persist.tile([68, T], F32R, tag=f"qaug{h}", name=f"qaug{h}") for h in range(HLOC)]
            k_aug = [persist.tile([68, T], F32R, tag=f"kaug{h}", name=f"kaug{h}") for h in range(HLOC)]
            v_sb = [
                persist.tile([128, HLOC * 65], BF16, tag=f"vsb{t}", name=f"vsb{t}") for t in range(JT)
            ]
            y_pack = [
                persist.tile([128, T], BF16, tag=f"ypk{p}", name=f"ypk{p}")
                for p in range(2)
            ]

            with ExitStack() as p2:
                xpool = p2.enter_context(tc.tile_pool(name="xT", bufs=1))
                wpool = p2.enter_context(tc.tile_pool(name="w", bufs=1))

                # interleaved load order: (wq, wk, x) per contraction chunk,
                # so the first projection matmul can start after ~3 DMAs.
                xT = []
                w_sb = {}
                for cc in range(KC):
                    wtq = wpool.tile([128, HD], BF16, tag=f"wq{cc}", name=f"wq{cc}")
                    nc.sync.dma_start(wtq[:], d_wq[128 * cc : 128 * cc + 128, :])
                    w_sb[("q", cc)] = wtq
                    wtk = wpool.tile([128, HD], BF16, tag=f"wk{cc}", name=f"wk{cc}")
                    nc.sync.dma_start(wtk[:], d_wk[128 * cc : 128 * cc + 128, :])
                    w_sb[("k", cc)] = wtk
                    t = xpool.tile([128, T], BF16, tag=f"xT{cc}", name=f"xT{cc}")
                    nc.sync.dma_start(t[:], d_xT[128 * cc : 128 * cc + 128, :])
                    xT.append(t)

                # consts
                stair01 = persist.tile([128, 128], BF16, tag="stair01", name="stair01")
                nc.sync.dma_start(stair01[:], d_stair01[:])
                ssqw_q = persist.tile([128, 97], F32R, tag="ssqwq", name="ssqwq")
                nc.sync.dma_start(ssqw_q[:], d_ssqw_q[:].bitcast(F32R))
                ssqw_k = persist.tile([128, 97], F32R, tag="ssqwk", name="ssqwk")
                nc.sync.dma_start(ssqw_k[:], d_ssqw_k[:].bitcast(F32R))
                sel_q = persist.tile([97, 128], F32R, tag="selq", name="selq")
                nc.sync.dma_start(sel_q[:], d_sel_q[:].bitcast(F32R))
                sel_k = persist.tile([97, 128], F32R, tag="selk", name="selk")
                nc.sync.dma_start(sel_k[:], d_sel_k[:].bitcast(F32R))
                ucol = persist.tile([D, 512], F32R, tag="ucol", name="ucol")
                nc.sync.dma_start(ucol[:], d_ucol[:].bitcast(F32R))
                omg97 = persist.tile([128, 1], F32, tag="omg97", name="omg97")
                nc.sync.dma_start(omg97[:], d_omg97[:])
                negomg97 = persist.tile([128, 1], F32, tag="negomg97", name="negomg97")
                nc.sync.dma_start(negomg97[:], d_negomg97[:])
                qw8 = persist.tile([128, 1], F32, tag="qw8", name="qw8")
                nc.sync.dma_start(qw8[:], d_qw8[:])
                kw = persist.tile([128, 1], F32, tag="kw", name="kw")
                nc.sync.dma_start(kw[:], d_kw[:])
                eps_col = persist.tile([128, 1], F32, tag="eps", name="eps")
                nc.vector.memset(eps_col[:], EPS_RMS)
                neghalf_col = persist.tile([128, 1], F32, tag="neghalf", name="neghalf")
                nc.vector.memset(neghalf_col[:], -0.5)
                one_col = persist.tile([128, 1], F32, tag="onec", name="onec")
                nc.vector.memset(one_col[:], 1.0)

                # q_aug fixed rows: 64/65 ones (for a_hi/a_lo), 66/67 iota
                for h in range(HLOC):
                    nc.sync.dma_start(q_aug[h][64:65, :], d_ones[:].bitcast(F32R))
                    nc.sync.dma_start(q_aug[h][65:66, :], d_ones[:].bitcast(F32R))
                    nc.sync.dma_start(q_aug[h][66:67, :], d_iota[:].bitcast(F32R))
                    nc.sync.dma_start(q_aug[h][67:68, :], d_iota[:].bitcast(F32R))

                # v ones columns (denominator trick)
                for t in range(JT):
                    dst = v_sb[t][:].rearrange("p (h d) -> p h d", h=HLOC)[:, :, 64:65]
                    nc.sync.dma_start(dst, d_ones4[:].rearrange("p (h o) -> p h o", o=1))

                # later-phase loads queued now so they stream in behind
                wv_sb = []
                for cc in range(KC):
                    wt = wpool.tile([128, HD], BF16, tag=f"wv{cc}", name=f"wv{cc}")
                    nc.sync.dma_start(wt[:], d_wv[128 * cc : 128 * cc + 128, :])
                    wv_sb.append(wt)
                wproj_sb = []
                for p in range(2):
                    wt = persist.tile([128, C], BF16, tag=f"wp{p}", name=f"wp{p}")
                    nc.sync.dma_start(wt[:], d_wproj[128 * p : 128 * p + 128, :])
                    wproj_sb.append(wt)
                iota97 = persist.tile([128, T], F32, tag="iota97", name="iota97")
                nc.sync.dma_start(iota97[:], d_iota97[:])

                # ============ P2: q/k projections, pipelined ============
                with ExitStack() as qk_stack:
                    qk_ps = qk_stack.enter_context(
                        tc.tile_pool(name="qkps", bufs=4, space="PSUM")
                    )
                    ssq_ps = qk_stack.enter_context(
                        tc.tile_pool(name="ssqps", bufs=1, space="PSUM")
                    )
                    rep_ps = qk_stack.enter_context(
                        tc.tile_pool(name="repps", bufs=2, space="PSUM")
                    )
                    sq_pool = qk_stack.enter_context(tc.tile_pool(name="qsq", bufs=4))
                    rsq_pool = qk_stack.enter_context(tc.tile_pool(name="rsq", bufs=2))
                    rep_sb_pool = qk_stack.enter_context(
                        tc.tile_pool(name="repsb", bufs=4)
                    )

                    steps = [(p, n) for p in range(2) for n in range(NT)]
                    ssq_fns = []   # deferred: ssq matmuls + ln/exp for step i
                    bc_fns = []    # deferred: broadcast matmuls + STT for step i

                    def emit_proj(name, p, n):
                        sl = slice(NCH * n, NCH * n + NCH)
                        ps = qk_ps.tile([128, NCH], F32, tag="qk", name=f"ps{name}")
                        for cc in range(KC):
                            nc.tensor.matmul(
                                ps[:],
                                w_sb[(name, cc)][:, 128 * p : 128 * p + 128],
                                xT[cc][:, sl],
                                start=(cc == 0),
                                stop=(cc == KC - 1),
                            )
                        sq = sq_pool.tile([128, NCH], F32R, tag=f"sq{name}", name=f"sq{name}")
                        nc.scalar.activation(sq[:], ps[:], AF.Square)
                        return ps, sq

                    def make_ssq_fn(qsq, ksq):
                        def fn():
                            ssq = ssq_ps.tile([97, NCH], F32, tag="ssq", name="ssq")
                            nc.tensor.matmul(
                                ssq[:], ssqw_q[:], qsq[:], start=True, stop=False
                            )
                            nc.tensor.matmul(
                                ssq[:], ssqw_k[:], ksq[:], start=False, stop=True
                            )
                            rsqa = rsq_pool.tile([97, NCH], F32R, tag="rsq", name="rsqa")
                            nc.scalar.activation(
                                rsqa[:], ssq[:], AF.Ln, bias=eps_col[0:97, :]
                            )
                            nc.scalar.activation(
                                rsqa[:], rsqa[:], AF.Exp, scale=neghalf_col[0:97, :]
                            )
                            return rsqa
                        return fn

                    def make_bc_fn(p, n, qps, kps, get_rsqa):
                        def fn():
                            sl = slice(NCH * n, NCH * n + NCH)
                            rsqa = get_rsqa()
                            rep_qp = rep_ps.tile([128, NCH], F32, tag="rep", name="repqp")
                            nc.tensor.matmul(
                                rep_qp[:], sel_q[:], rsqa[:], start=True, stop=True
                            )
                            rep_kp = rep_ps.tile([128, NCH], F32, tag="rep", name="repkp")
                            nc.tensor.matmul(
                                rep_kp[:], sel_k[:], rsqa[:], start=True, stop=True
                            )
                            # STT's SBUF input must sit at the output's base
                            # partition: stage each slot at base 0 via ACT.
                            reps = {}
                            for name, rp in (("q", rep_qp), ("k", rep_kp)):
                                for s in range(2):
                                    rt = rep_sb_pool.tile(
                                        [64, NCH], F32, tag="repsb", name=f"rep{name}{s}"
                                    )
                                    nc.scalar.activation(
                                        rt[:], rp[64 * s : 64 * s + 64, :], AF.Copy
                                    )
                                    reps[(name, s)] = rt
                            for s in range(2):
                                h = 2 * p + s
                                rows = slice(64 * s, 64 * s + 64)
                                nc.vector.scalar_tensor_tensor(
                                    q_aug[h][0:64, sl],
                                    qps[rows, :],
                                    qw8[0:64, :],
                                    reps[("q", s)][:],
                                    MUL,
                                    MUL,
                                )
                                nc.vector.scalar_tensor_tensor(
                                    k_aug[h][0:64, sl],
                                    kps[rows, :],
                                    kw[0:64, :],
                                    reps[("k", s)][:],
                                    MUL,
                                    MUL,
                                )
                        return fn

                    rsq_box = {}
                    for i, (p, n) in enumerate(steps):
                        qps, qsq = emit_proj("q", p, n)
                        if ssq_fns:
                            fn = ssq_fns.pop(0)
                            rsq_box[i - 1] = fn()
                        kps, ksq = emit_proj("k", p, n)
                        if bc_fns:
                            bc_fns.pop(0)()
                        ssq_fns.append(make_ssq_fn(qsq, ksq))
                        bc_fns.append(
                            make_bc_fn(p, n, qps, kps,
                                       (lambda j: (lambda: rsq_box[j]))(i))
                        )
                    # tail flush
                    rsq_box[len(steps) - 1] = ssq_fns.pop(0)()
                    bc_fns.pop(0)()

                # ============ v projection ============
                with ExitStack() as v_stack:
                    v_ps_pool = v_stack.enter_context(
                        tc.tile_pool(name="vps", bufs=2, space="PSUM")
                    )
                    glog_ps = v_stack.enter_context(
                        tc.tile_pool(name="glps", bufs=2, space="PSUM")
                    )
                    p3_pool = v_stack.enter_context(tc.tile_pool(name="p3", bufs=1))

                    for t in range(JT):
                        vps = v_ps_pool.tile([128, HD], F32, tag="vps", name="vps")
                        for cc in range(KC):
                            nc.tensor.matmul(
                                vps[:],
                                xT[cc][:, 128 * t : 128 * t + 128],
                                wv_sb[cc][:],
                                start=(cc == 0),
                                stop=(cc == KC - 1),
                            )
                        dst = v_sb[t][:].rearrange("p (h d) -> p h d", h=HLOC)[:, :, 0:64]
                        nc.vector.tensor_copy(
                            dst, vps[:].rearrange("p (h d) -> p h d", h=HLOC)
                        )

                    # ============ P3: key gate / bias rows ============
                    gsc_wide = p3_pool.tile([128, T], F32R, tag="gsc", name="gsc")
                    for n in range(NT):
                        sl = slice(NCH * n, NCH * n + NCH)
                        glsp = glog_ps.tile([128, NCH], F32, tag="glsp", name="glsp")
                        for h in range(HLOC):
                            nc.tensor.matmul(
                                glsp[:],
                                ucol[:, 128 * h : 128 * h + 128],
                                k_aug[h][0:64, sl],
                                start=(h == 0),
                                stop=(h == HLOC - 1),
                            )
                        nc.scalar.activation(gsc_wide[:, sl], glsp[:], AF.Exp)

                    gate4 = p3_pool.tile([128, T], F32, tag="gate4", name="gate4")
                    nc.scalar.activation(
                        gate4[:], gsc_wide[:], AF.Ln, bias=one_col[:]
                    )
                    a4f = p3_pool.tile([128, T], F32, tag="a4f", name="a4f")
                    nc.vector.scalar_tensor_tensor(
                        a4f[:], gate4[:], omg97[:], iota97[:], MUL, MUL
                    )
                    w4f = p3_pool.tile([128, T], F32, tag="w4f", name="w4f")
                    nc.vector.tensor_scalar_mul(w4f[:], gate4[:], negomg97[:])
                    a_hi = p3_pool.tile([128, T], F32R, tag="a_hi", name="a_hi")
                    nc.vector.tensor_copy(a_hi[:], a4f[:])
                    a_lo = p3_pool.tile([128, T], F32R, tag="a_lo", name="a_lo")
                    nc.vector.scalar_tensor_tensor(
                        a_lo[:], a4f[:], 1.0, a_hi[:].bitcast(F32), MUL, SUB
                    )
                    w_hi = p3_pool.tile([128, T], F32R, tag="w_hi", name="w_hi")
                    nc.vector.tensor_copy(w_hi[:], w4f[:])
                    w_lo = p3_pool.tile([128, T], F32R, tag="w_lo", name="w_lo")
                    nc.vector.scalar_tensor_tensor(
                        w_lo[:], w4f[:], 1.0, w_hi[:].bitcast(F32), MUL, SUB
                    )
                    for h in range(HLOC):
                        r = 32 * h
                        nc.sync.dma_start(k_aug[h][64:65, :], a_hi[r : r + 1, :])
                        nc.sync.dma_start(k_aug[h][65:66, :], a_lo[r : r + 1, :])
                        nc.sync.dma_start(k_aug[h][66:67, :], w_hi[r : r + 1, :])
                        nc.sync.dma_start(k_aug[h][67:68, :], w_lo[r : r + 1, :])

            # ============ P4 + P5: attention & projection ============
            with ExitStack() as p4:
                pair_ps = p4.enter_context(
                    tc.tile_pool(name="pairps", bufs=2, space="PSUM")
                )
                y_ps_pool = p4.enter_context(
                    tc.tile_pool(name="yps", bufs=2, space="PSUM")
                )
                o_ps_pool = p4.enter_context(
                    tc.tile_pool(name="ops", bufs=2, space="PSUM")
                )
                psb_pool = p4.enter_context(tc.tile_pool(name="psb", bufs=3))
                den_pool = p4.enter_context(tc.tile_pool(name="den", bufs=2))
                rep2_pool = p4.enter_context(tc.tile_pool(name="rep2", bufs=2))
                out_pool = p4.enter_context(tc.tile_pool(name="osb", bufs=2))
                dram2 = p4.enter_context(
                    tc.tile_pool(name="dram2", bufs=4, space="DRAM")
                )

                cproj_queue = []

                def emit_cproj_unit(tt, cn):
                    osl = slice(512 * cn, 512 * cn + 512)
                    ops = o_ps_pool.tile([128, 512], F32, tag="ops", name="ops")
                    for p in range(2):
                        nc.tensor.matmul(
                            ops[:],
                            y_pack[p][:, 128 * tt : 128 * tt + 128],
                            wproj_sb[p][:, osl],
                            start=(p == 0),
                            stop=(p == 1),
                        )
                    osb = out_pool.tile([128, 512], F32, tag="osb", name="osb")
                    nc.vector.tensor_copy(osb[:], ops[:])
                    nc.sync.dma_start(d_out[128 * tt : 128 * tt + 128, osl], osb[:])

                for ci in range(NT):
                    isl = slice(NCH * ci, NCH * ci + NCH)
                    for h in range(HLOC):
                        p_pk, s_slot = divmod(h, 2)
                        yps = y_ps_pool.tile([65, NCH], F32, tag="yps", name="yps")
                        nblocks = 4 * ci + 4
                        # groups of 2 blocks: (tj, off) pairs
                        groups = []
                        tj = 0
                        while tj < nblocks:
                            r0, r1 = tj - 4 * ci, tj + 1 - 4 * ci
                            groups.append(
                                (tj, max(0, 128 * r0), tj + 1, max(0, 128 * r1),
                                 r0 >= 0)
                            )
                            tj += 2

                        def emit_scores(g):
                            t0, off0, t1, off1, _diag = g
                            pair = pair_ps.tile(
                                [128, 2 * NCH], F32, tag="pair", name="pair"
                            )
                            nc.tensor.matmul(
                                pair[:, off0:NCH],
                                k_aug[h][:, 128 * t0 : 128 * t0 + 128],
                                q_aug[h][:, NCH * ci + off0 : NCH * (ci + 1)],
                                start=True,
                                stop=True,
                                skip_group_check=True,
                            )
                            nc.tensor.matmul(
                                pair[:, NCH + off1 : 2 * NCH],
                                k_aug[h][:, 128 * t1 : 128 * t1 + 128],
                                q_aug[h][:, NCH * ci + off1 : NCH * (ci + 1)],
                                start=True,
                                stop=True,
                                skip_group_check=True,
                            )
                            return pair

                        def emit_tail(g, pair, first, last):
                            t0, off0, t1, off1, diag = g
                            psb = psb_pool.tile(
                                [128, 2 * NCH], BF16, tag="psb", name="psb"
                            )
                            if diag:
                                nc.scalar.activation(
                                    psb[:, off0:NCH], pair[:, off0:NCH], AF.Exp
                                )
                                nc.scalar.activation(
                                    psb[:, NCH + off1 : 2 * NCH],
                                    pair[:, NCH + off1 : 2 * NCH],
                                    AF.Exp,
                                )
                                # causal wedge mask: keep where j<=i
                                nc.vector.tensor_tensor(
                                    psb[:, off0 : off0 + 128],
                                    psb[:, off0 : off0 + 128],
                                    stair01[:],
                                    MIN,
                                )
                                nc.vector.tensor_tensor(
                                    psb[:, NCH + off1 : NCH + off1 + 128],
                                    psb[:, NCH + off1 : NCH + off1 + 128],
                                    stair01[:],
                                    MIN,
                                )
                            else:
                                nc.scalar.activation(
                                    psb[:], pair[:], AF.Exp
                                )
                            nc.tensor.matmul(
                                yps[:, off0:NCH],
                                v_sb[t0][:, 65 * h : 65 * h + 65],
                                psb[:, off0:NCH],
                                start=first,
                                stop=False,
                                skip_group_check=True,
                            )
                            nc.tensor.matmul(
                                yps[:, off1:NCH],
                                v_sb[t1][:, 65 * h : 65 * h + 65],
                                psb[:, NCH + off1 : 2 * NCH],
                                start=False,
                                stop=last,
                                skip_group_check=True,
                            )

                        prev = None
                        for gi, g in enumerate(groups):
                            pair = emit_scores(g)
                            if prev is not None:
                                pg, ppair, pfirst = prev
                                emit_tail(pg, ppair, pfirst, False)
                                if cproj_queue:
                                    cproj_queue.pop(0)()
                            prev = (g, pair, gi == 0)
                        pg, ppair, pfirst = prev
                        emit_tail(pg, ppair, pfirst, True)
                        if cproj_queue:
                            cproj_queue.pop(0)()

                        # normalize: 1/denom via DVE, broadcast via DRAM
                        den = den_pool.tile([65, NCH], F32, tag="den", name="den")
                        nc.vector.reciprocal(den[64:65, :], yps[64:65, :])
                        drow = dram2.tile([1, NCH], F32, tag="dden", name="dden")
                        nc.sync.dma_start(drow[:], den[64:65, :])
                        rrep = rep2_pool.tile([64, NCH], F32, tag="rrep", name="rrep")
                        nc.sync.dma_start(
                            rrep[:], drow[0:1, :].partition_broadcast(64)
                        )
                        nc.vector.tensor_tensor(
                            y_pack[p_pk][64 * s_slot : 64 * s_slot + 64, isl],
                            yps[0:64, :],
                            rrep[:],
                            MUL,
                        )
                    # queue this chunk's projection; emitted during next chunk
                    for tt in range(4 * ci, 4 * ci + 4):
                        for cn in range(2):
                            cproj_queue.append(
                                (lambda a, b: (lambda: emit_cproj_unit(a, b)))(tt, cn)
                            )
                while cproj_queue:
                    cproj_queue.pop(0)()

    split_excess_waits(nc, max_waits=1)
    return nc


def _host_shards(inputs):
    x = np.asarray(inputs["x"], np.float32)
    Wq = np.asarray(inputs["Wq"], np.float32)
    Wk = np.asarray(inputs["Wk"], np.float32)
    Wv = np.asarray(inputs["Wv"], np.float32)
    Wproj = np.asarray(inputs["Wproj"], np.float32)
    q_rms_w = np.asarray(inputs["q_rms_w"], np.float32)
    k_rms_w = np.asarray(inputs["k_rms_w"], np.float32)
    omega = np.asarray(inputs["omega"], np.float32)
    u = np.asarray(inputs["u"], np.float32)

    import ml_dtypes

    bf16 = ml_dtypes.bfloat16
    slopes = np.asarray(_get_alibi_slopes(H), np.float32)
    omega_eff = np.log1p(np.exp(omega)) * slopes  # softplus(omega) * slopes
    u_n = u / np.maximum(
        np.linalg.norm(u, axis=-1, keepdims=True), U_L2_EPS
    )
    sqrt_d = math.sqrt(D)

    iota = np.arange(T, dtype=np.float32)[None, :]
    ones_row = np.ones((1, T), np.float32)
    ones4 = np.ones((128, HLOC), np.float32).astype(bf16)
    jj = np.arange(128, dtype=np.float32)
    # MIN-mask: keep (BIG) where j<=i, zero where j>i
    stair01 = np.where(jj[:, None] <= jj[None, :], BIGF, 0.0).astype(bf16)
    ssqw_q = np.zeros((128, 97), np.float32)
    ssqw_q[0:64, 0] = 1.0 / D
    ssqw_q[64:128, 32] = 1.0 / D
    ssqw_k = np.zeros((128, 97), np.float32)
    ssqw_k[0:64, 64] = 1.0 / D
    ssqw_k[64:128, 96] = 1.0 / D
    sel_q = np.zeros((97, 128), np.float32)
    sel_q[0, 0:64] = 1.0
    sel_q[32, 64:128] = 1.0
    sel_k = np.zeros((97, 128), np.float32)
    sel_k[64, 0:64] = 1.0
    sel_k[96, 64:128] = 1.0
    qw8 = np.tile(q_rms_w / 8.0, 2)[:, None].astype(np.float32)
    kw = np.tile(k_rms_w, 2)[:, None].astype(np.float32)
    iota97 = np.tile(iota, (128, 1)).astype(np.float32)

    in_maps = []
    for core in range(8):
        b, g = divmod(core, HLOC)
        hs = slice(HLOC * g, HLOC * g + HLOC)
        cs = slice(HD * g, HD * g + HD)
        ucol_ext = np.zeros((D, 512), np.float32)
        for h in range(HLOC):
            ucol_ext[:, 128 * h + 32 * h] = u_n[HLOC * g + h] / sqrt_d
        omg97 = np.zeros((128, 1), np.float32)
        negomg97 = np.zeros((128, 1), np.float32)
        for h in range(HLOC):
            omg97[32 * h, 0] = omega_eff[HLOC * g + h]
            negomg97[32 * h, 0] = -omega_eff[HLOC * g + h]
        in_maps.append(
            {
                "xT": np.ascontiguousarray(x[b].T).astype(bf16),
                "wq": np.ascontiguousarray(Wq[:, cs]).astype(bf16),
                "wk": np.ascontiguousarray(Wk[:, cs]).astype(bf16),
                "wv": np.ascontiguousarray(Wv[:, cs]).astype(bf16),
                "wproj": np.ascontiguousarray(Wproj[cs, :]).astype(bf16),
                "ssqw_q": ssqw_q,
                "ssqw_k": ssqw_k,
                "sel_q": sel_q,
                "sel_k": sel_k,
                "ucol": ucol_ext,
                "omg97": omg97,
                "negomg97": negomg97,
                "iota97": iota97,
                "iota": iota,
                "ones_row": ones_row,
                "ones4": ones4,
                "stair01": stair01,
                "qw8": qw8,
                "kw": kw,
            }
        )
    return in_maps


def kernel(**inputs):
    from concourse.bass_utils import run_bass_kernel_spmd

    if "nc" not in _cache:
        _cache["nc"] = _build_program()
    nc = _cache["nc"]

    in_maps = _host_shards(inputs)
    res = run_bass_kernel_spmd(nc, in_maps, core_ids=list(range(8)))
    out = np.zeros((B, T, C), np.float32)
    for core in range(8):
        b = core // HLOC
        out[b] += res.results[core]["out"]
    return out


# revision 13
# speedup vs baseline: 1.4903x; 1.0103x over previous
"""Causal self-attention (RMSNorm QK, key-gated ALiBi bias) on 8 TRN2 cores.

Sharding: data-parallel over batch (2) x tensor-parallel over heads (4 groups
of 4 heads) = 8 cores. Each core computes a partial c_proj output for its
batch; the host sums the 4 head-group partials per batch.

Device kernel (identical SPMD program, per-core data):
  P2  QKV projections (bf16 matmuls, xT bf16 resident in SBUF).
      Per (pack p, chunk n): q/k PSUM -> Square(ACT) -> sum-sq matmul into a
      partition-spread [97,512] PSUM ({0,32,64,96} rows) -> ln/exp rsqrt ->
      matmul-broadcast (sel x rsq) to [128,512] PSUM -> DVE STT writes
      normalized q/k into per-head augmented tiles [68,T] f32r.
      Software-pipelined so the in-order PE never waits on ACT.
  P3  key-gate: per-head gate logits matmul into partition-spread PSUM rows,
      one exp per chunk, one wide ln, wide DVE ops build a=j*w / w rows with
      f32r hi/lo splits, DMA'd into k_aug rows 64..67.
  P4  scores^T = k_aug . q_aug in [128,1024] PSUM pairs (bias folded into the
      contraction), one exp per pair (ACT), causal wedge masked via bf16 MIN
      on the diagonal blocks (DVE 4x), PV matmul with a ones-column in v
      giving softmax denominators free.
  P5  y normalized via DVE reciprocal + DRAM-roundtrip broadcast, packed
      2 heads/tile bf16, c_proj matmul interleaved into the next chunk's
      attention, partial output to DRAM.
"""

import sys

if "/opt/trn_rl_repo" not in sys.path:
    sys.path.insert(0, "/opt/trn_rl_repo")

import math

import numpy as np

B, T, C = 2, 2048, 1024
H, D = 16, 64
HLOC = 4           # heads per core
HD = HLOC * D      # 256
NCH = 512          # i-chunk width
NT = T // NCH      # 4 i-chunks
JT = T // 128      # 16 j-tiles
KC = C // 128      # 8 contraction chunks
EPS_RMS = 1e-5
U_L2_EPS = 1e-6
BIGF = 3.0e38      # bf16-representable "+inf" for the MIN mask

_cache = {}


def _get_alibi_slopes(n_heads):
    def pow2(n):
        start = 2 ** (-(2 ** (-(math.log2(n) - 3))))
        return [start * start**i for i in range(n)]

    if math.log2(n_heads).is_integer():
        return pow2(n_heads)
    c = 2 ** math.floor(math.log2(n_heads))
    s = pow2(c)
    extra = _get_alibi_slopes(2 * c)
    return s + extra[0::2][: n_heads - c]


def _build_program():
    import concourse.bass as bass
    import concourse.mybir as mybir
    import concourse.tile as tile
    from concourse.alu_op_type import AluOpType
    from concourse.vector_clock import ScopedClock

    F32 = mybir.dt.float32
    F32R = mybir.dt.float32r
    BF16 = mybir.dt.bfloat16
    AF = mybir.ActivationFunctionType
    MUL = AluOpType.mult
    ADD = AluOpType.add
    SUB = AluOpType.subtract
    MIN = AluOpType.min

    class PatchedTileContext(tile.TileContext):
        """Tail drain split into nops carrying <=2 sem waits each (this
        walrus build rejects CTRL instructions with more)."""

        def _drain_and_barrier(self, tick_clock, wait_clock):
            nc = self.nc
            probe = nc.sync.nop(nofuse=True)
            wait_clock.add_sem_waits(
                probe.ins, ScopedClock({None: tick_clock.global_clock})
            )
            si = probe.ins.sync_info
            waits = list(si.on_wait or []) if si is not None else []
            if len(waits) > 2:
                si.on_wait = waits[:2]
                rest = waits[2:]
                for i in range(0, len(rest), 2):
                    extra = nc.sync.nop(nofuse=True)
                    esi = extra.ins.sync_info
                    chunk = rest[i : i + 2]
                    if esi is None:
                        extra.ins.sync_info = mybir.SyncInfo(
                            on_wait=chunk, on_update=[]
                        )
                    else:
                        esi.on_wait = (esi.on_wait or []) + chunk
            nc.sync.drain()
            nc.all_engine_barrier()
            assert self.sems is not None
            popped = nc._tile_sem_poison_stack.pop()
            assert popped is self._sem_poison
            nc.clear_and_free_semaphores(list(self.sems.allocated().values()))
            nc.all_engine_barrier()

    def split_excess_waits(nc, max_waits=1):
        for f in nc.m.functions:
            for blk in f.blocks:
                new_insts = []
                for inst in blk.instructions:
                    si = inst.sync_info
                    if si is not None and si.on_wait and len(si.on_wait) > max_waits:
                        waits = list(si.on_wait)
                        si.on_wait = waits[-max_waits:]
                        rest = waits[:-max_waits]
                        for i in range(0, len(rest), max_waits):
                            nop = mybir.InstNoOp(
                                name=f"I-waitsplit-{nc.next_id()}",
                                ins=[],
                                outs=[],
                                engine=inst.engine,
                                sync_info=mybir.SyncInfo(
                                    on_wait=rest[i : i + max_waits], on_update=[]
                                ),
                            )
                            nc.register_instruction(nop)
                            new_insts.append(nop)
                    new_insts.append(inst)
                blk.instructions = new_insts

    nc = bass.Bass(trn_type="TRN2", num_devices=8, debug=False)

    # ---- DRAM I/O (per-core shards supplied by the host) ----
    d_xT = nc.dram_tensor("xT", [C, T], BF16, kind="ExternalInput")
    d_wq = nc.dram_tensor("wq", [C, HD], BF16, kind="ExternalInput")
    d_wk = nc.dram_tensor("wk", [C, HD], BF16, kind="ExternalInput")
    d_wv = nc.dram_tensor("wv", [C, HD], BF16, kind="ExternalInput")
    d_wproj = nc.dram_tensor("wproj", [HD, C], BF16, kind="ExternalInput")
    d_ssqw_q = nc.dram_tensor("ssqw_q", [128, 97], F32, kind="ExternalInput")
    d_ssqw_k = nc.dram_tensor("ssqw_k", [128, 97], F32, kind="ExternalInput")
    d_sel_q = nc.dram_tensor("sel_q", [97, 128], F32, kind="ExternalInput")
    d_sel_k = nc.dram_tensor("sel_k", [97, 128], F32, kind="ExternalInput")
    d_ucol = nc.dram_tensor("ucol", [D, 512], F32, kind="ExternalInput")
    d_omg97 = nc.dram_tensor("omg97", [128, 1], F32, kind="ExternalInput")
    d_negomg97 = nc.dram_tensor("negomg97", [128, 1], F32, kind="ExternalInput")
    d_iota97 = nc.dram_tensor("iota97", [128, T], F32, kind="ExternalInput")
    d_iota = nc.dram_tensor("iota", [1, T], F32, kind="ExternalInput")
    d_ones = nc.dram_tensor("ones_row", [1, T], F32, kind="ExternalInput")
    d_ones4 = nc.dram_tensor("ones4", [128, HLOC], BF16, kind="ExternalInput")
    d_stairT = nc.dram_tensor("stairT", [128, 128], BF16, kind="ExternalInput")
    d_stairmov = nc.dram_tensor("stairmov", [128, 512], BF16, kind="ExternalInput")
    d_qw8 = nc.dram_tensor("qw8", [128, 1], F32, kind="ExternalInput")
    d_kw = nc.dram_tensor("kw", [128, 1], F32, kind="ExternalInput")
    d_out = nc.dram_tensor("out", [T, C], BF16, kind="ExternalOutput")

    with PatchedTileContext(nc) as tc:
        from contextlib import ExitStack

        with ExitStack() as top:
            persist = top.enter_context(tc.tile_pool(name="persist", bufs=1))

            # ---- persistent SBUF tensors ----
            q_aug = [persist.tile([68, T], F32R, tag=f"qaug{h}", name=f"qaug{h}") for h in range(HLOC)]
            k_aug = [persist.tile([68, T], F32R, tag=f"kaug{h}", name=f"kaug{h}") for h in range(HLOC)]
            v_sb = [
                persist.tile([128, HLOC * 65], BF16, tag=f"vsb{t}", name=f"vsb{t}") for t in range(JT)
            ]
            y_pack = [
                persist.tile([128, T], BF16, tag=f"ypk{p}", name=f"ypk{p}")
                for p in range(2)
            ]

            with ExitStack() as p2:
                xpool = p2.enter_context(tc.tile_pool(name="xT", bufs=1))
                wpool = p2.enter_context(tc.tile_pool(name="w", bufs=1))

                # interleaved load order: (wq, wk, x) per contraction chunk,
                # so the first projection matmul can start after ~3 DMAs.
                xT = []
                w_sb = {}
                for cc in range(KC):
                    wtq = wpool.tile([128, HD], BF16, tag=f"wq{cc}", name=f"wq{cc}")
                    nc.sync.dma_start(wtq[:], d_wq[128 * cc : 128 * cc + 128, :])
                    w_sb[("q", cc)] = wtq
                    wtk = wpool.tile([128, HD], BF16, tag=f"wk{cc}", name=f"wk{cc}")
                    nc.sync.dma_start(wtk[:], d_wk[128 * cc : 128 * cc + 128, :])
                    w_sb[("k", cc)] = wtk
                    t = xpool.tile([128, T], BF16, tag=f"xT{cc}", name=f"xT{cc}")
                    nc.sync.dma_start(t[:], d_xT[128 * cc : 128 * cc + 128, :])
                    xT.append(t)

                # consts
                stairT = persist.tile([128, 128], BF16, tag="stairT", name="stairT")
                nc.sync.dma_start(stairT[:], d_stairT[:])
                stairmov = persist.tile([128, 512], BF16, tag="stairmov", name="stairmov")
                nc.sync.dma_start(stairmov[:], d_stairmov[:])
                ssqw_q = persist.tile([128, 97], F32R, tag="ssqwq", name="ssqwq")
                nc.sync.dma_start(ssqw_q[:], d_ssqw_q[:].bitcast(F32R))
                ssqw_k = persist.tile([128, 97], F32R, tag="ssqwk", name="ssqwk")
                nc.sync.dma_start(ssqw_k[:], d_ssqw_k[:].bitcast(F32R))
                sel_q = persist.tile([97, 128], F32R, tag="selq", name="selq")
                nc.sync.dma_start(sel_q[:], d_sel_q[:].bitcast(F32R))
                sel_k = persist.tile([97, 128], F32R, tag="selk", name="selk")
                nc.sync.dma_start(sel_k[:], d_sel_k[:].bitcast(F32R))
                ucol = persist.tile([D, 512], F32R, tag="ucol", name="ucol")
                nc.sync.dma_start(ucol[:], d_ucol[:].bitcast(F32R))
                omg97 = persist.tile([128, 1], F32, tag="omg97", name="omg97")
                nc.sync.dma_start(omg97[:], d_omg97[:])
                negomg97 = persist.tile([128, 1], F32, tag="negomg97", name="negomg97")
                nc.sync.dma_start(negomg97[:], d_negomg97[:])
                qw8 = persist.tile([128, 1], F32, tag="qw8", name="qw8")
                nc.sync.dma_start(qw8[:], d_qw8[:])
                kw = persist.tile([128, 1], F32, tag="kw", name="kw")
                nc.sync.dma_start(kw[:], d_kw[:])
                eps_col = persist.tile([128, 1], F32, tag="eps", name="eps")
                nc.vector.memset(eps_col[:], EPS_RMS)
                neghalf_col = persist.tile([128, 1], F32, tag="neghalf", name="neghalf")
                nc.vector.memset(neghalf_col[:], -0.5)
                one_col = persist.tile([128, 1], F32, tag="onec", name="onec")
                nc.vector.memset(one_col[:], 1.0)

                # q_aug fixed rows: 64/65 ones (for a_hi/a_lo), 66/67 iota
                for h in range(HLOC):
                    nc.sync.dma_start(q_aug[h][64:65, :], d_ones[:].bitcast(F32R))
                    nc.sync.dma_start(q_aug[h][65:66, :], d_ones[:].bitcast(F32R))
                    nc.sync.dma_start(q_aug[h][66:67, :], d_iota[:].bitcast(F32R))
                    nc.sync.dma_start(q_aug[h][67:68, :], d_iota[:].bitcast(F32R))

                # v ones columns (denominator trick)
                for t in range(JT):
                    dst = v_sb[t][:].rearrange("p (h d) -> p h d", h=HLOC)[:, :, 64:65]
                    nc.sync.dma_start(dst, d_ones4[:].rearrange("p (h o) -> p h o", o=1))

                # later-phase loads queued now so they stream in behind
                wv_sb = []
                for cc in range(KC):
                    wt = wpool.tile([128, HD], BF16, tag=f"wv{cc}", name=f"wv{cc}")
                    nc.sync.dma_start(wt[:], d_wv[128 * cc : 128 * cc + 128, :])
                    wv_sb.append(wt)
                wproj_sb = []
                for p in range(2):
                    wt = persist.tile([128, C], BF16, tag=f"wp{p}", name=f"wp{p}")
                    nc.sync.dma_start(wt[:], d_wproj[128 * p : 128 * p + 128, :])
                    wproj_sb.append(wt)
                iota97 = persist.tile([128, T], F32, tag="iota97", name="iota97")
                nc.sync.dma_start(iota97[:], d_iota97[:])

                # ============ P2: q/k projections, pipelined ============
                with ExitStack() as qk_stack:
                    qk_ps = qk_stack.enter_context(
                        tc.tile_pool(name="qkps", bufs=4, space="PSUM")
                    )
                    ssq_ps = qk_stack.enter_context(
                        tc.tile_pool(name="ssqps", bufs=1, space="PSUM")
                    )
                    rep_ps = qk_stack.enter_context(
                        tc.tile_pool(name="repps", bufs=2, space="PSUM")
                    )
                    sq_pool = qk_stack.enter_context(tc.tile_pool(name="qsq", bufs=4))
                    rsq_pool = qk_stack.enter_context(tc.tile_pool(name="rsq", bufs=2))
                    rep_sb_pool = qk_stack.enter_context(
                        tc.tile_pool(name="repsb", bufs=4)
                    )

                    steps = [(p, n) for p in range(2) for n in range(NT)]
                    ssq_fns = []   # deferred: ssq matmuls + ln/exp for step i
                    bc_fns = []    # deferred: broadcast matmuls + STT for step i

                    def emit_proj(name, p, n):
                        sl = slice(NCH * n, NCH * n + NCH)
                        ps = qk_ps.tile([128, NCH], F32, tag="qk", name=f"ps{name}")
                        for cc in range(KC):
                            nc.tensor.matmul(
                                ps[:],
                                w_sb[(name, cc)][:, 128 * p : 128 * p + 128],
                                xT[cc][:, sl],
                                start=(cc == 0),
                                stop=(cc == KC - 1),
                            )
                        sq = sq_pool.tile([128, NCH], F32R, tag=f"sq{name}", name=f"sq{name}")
                        nc.scalar.activation(sq[:], ps[:], AF.Square)
                        return ps, sq

                    def make_ssq_fn(qsq, ksq):
                        def fn():
                            ssq = ssq_ps.tile([97, NCH], F32, tag="ssq", name="ssq")
                            nc.tensor.matmul(
                                ssq[:], ssqw_q[:], qsq[:], start=True, stop=False
                            )
                            nc.tensor.matmul(
                                ssq[:], ssqw_k[:], ksq[:], start=False, stop=True
                            )
                            rsqa = rsq_pool.tile([97, NCH], F32R, tag="rsq", name="rsqa")
                            nc.scalar.activation(
                                rsqa[:], ssq[:], AF.Ln, bias=eps_col[0:97, :]
                            )
                            nc.scalar.activation(
                                rsqa[:], rsqa[:], AF.Exp, scale=neghalf_col[0:97, :]
                            )
                            return rsqa
                        return fn

                    def make_bc_fn(p, n, qps, kps, get_rsqa):
                        def fn():
                            sl = slice(NCH * n, NCH * n + NCH)
                            rsqa = get_rsqa()
                            rep_qp = rep_ps.tile([128, NCH], F32, tag="rep", name="repqp")
                            nc.tensor.matmul(
                                rep_qp[:], sel_q[:], rsqa[:], start=True, stop=True
                            )
                            rep_kp = rep_ps.tile([128, NCH], F32, tag="rep", name="repkp")
                            nc.tensor.matmul(
                                rep_kp[:], sel_k[:], rsqa[:], start=True, stop=True
                            )
                            # STT's SBUF input must sit at the output's base
                            # partition: stage each slot at base 0 via ACT.
                            reps = {}
                            for name, rp in (("q", rep_qp), ("k", rep_kp)):
                                for s in range(2):
                                    rt = rep_sb_pool.tile(
                                        [64, NCH], F32, tag="repsb", name=f"rep{name}{s}"
                                    )
                                    nc.scalar.activation(
                                        rt[:], rp[64 * s : 64 * s + 64, :], AF.Copy
                                    )
                                    reps[(name, s)] = rt
                            for s in range(2):
                                h = 2 * p + s
                                rows = slice(64 * s, 64 * s + 64)
                                nc.vector.scalar_tensor_tensor(
                                    q_aug[h][0:64, sl],
                                    qps[rows, :],
                                    qw8[0:64, :],
                                    reps[("q", s)][:],
                                    MUL,
                                    MUL,
                                )
                                nc.vector.scalar_tensor_tensor(
                                    k_aug[h][0:64, sl],
                                    kps[rows, :],
                                    kw[0:64, :],
                                    reps[("k", s)][:],
                                    MUL,
                                    MUL,
                                )
                        return fn

                    rsq_box = {}
                    for i, (p, n) in enumerate(steps):
                        qps, qsq = emit_proj("q", p, n)
                        if ssq_fns:
                            fn = ssq_fns.pop(0)
                            rsq_box[i - 1] = fn()
                        kps, ksq = emit_proj("k", p, n)
                        if bc_fns:
                            bc_fns.pop(0)()
                        ssq_fns.append(make_ssq_fn(qsq, ksq))
                        bc_fns.append(
                            make_bc_fn(p, n, qps, kps,
                                       (lambda j: (lambda: rsq_box[j]))(i))
                        )
                    # tail flush
                    rsq_box[len(steps) - 1] = ssq_fns.pop(0)()
                    bc_fns.pop(0)()

                # ============ v projection ============
                with ExitStack() as v_stack:
                    v_ps_pool = v_stack.enter_context(
                        tc.tile_pool(name="vps", bufs=2, space="PSUM")
                    )
                    glog_ps = v_stack.enter_context(
                        tc.tile_pool(name="glps", bufs=2, space="PSUM")
                    )
                    p3_pool = v_stack.enter_context(tc.tile_pool(name="p3", bufs=1))

                    for t in range(JT):
                        vps = v_ps_pool.tile([128, HD], F32, tag="vps", name="vps")
                        for cc in range(KC):
                            nc.tensor.matmul(
                                vps[:],
                                xT[cc][:, 128 * t : 128 * t + 128],
                                wv_sb[cc][:],
                                start=(cc == 0),
                                stop=(cc == KC - 1),
                            )
                        dst = v_sb[t][:].rearrange("p (h d) -> p h d", h=HLOC)[:, :, 0:64]
                        nc.vector.tensor_copy(
                            dst, vps[:].rearrange("p (h d) -> p h d", h=HLOC)
                        )

                    # ============ P3: key gate / bias rows ============
                    gsc_wide = p3_pool.tile([128, T], F32R, tag="gsc", name="gsc")
                    for n in range(NT):
                        sl = slice(NCH * n, NCH * n + NCH)
                        glsp = glog_ps.tile([128, NCH], F32, tag="glsp", name="glsp")
                        for h in range(HLOC):
                            nc.tensor.matmul(
                                glsp[:],
                                ucol[:, 128 * h : 128 * h + 128],
                                k_aug[h][0:64, sl],
                                start=(h == 0),
                                stop=(h == HLOC - 1),
                            )
                        nc.scalar.activation(gsc_wide[:, sl], glsp[:], AF.Exp)

                    gate4 = p3_pool.tile([128, T], F32, tag="gate4", name="gate4")
                    nc.scalar.activation(
                        gate4[:], gsc_wide[:], AF.Ln, bias=one_col[:]
                    )
                    a4f = p3_pool.tile([128, T], F32, tag="a4f", name="a4f")
                    nc.vector.scalar_tensor_tensor(
                        a4f[:], gate4[:], omg97[:], iota97[:], MUL, MUL
                    )
                    w4f = p3_pool.tile([128, T], F32, tag="w4f", name="w4f")
                    nc.vector.tensor_scalar_mul(w4f[:], gate4[:], negomg97[:])
                    a_hi = p3_pool.tile([128, T], F32R, tag="a_hi", name="a_hi")
                    nc.vector.tensor_copy(a_hi[:], a4f[:])
                    a_lo = p3_pool.tile([128, T], F32R, tag="a_lo", name="a_lo")
                    nc.vector.scalar_tensor_tensor(
                        a_lo[:], a4f[:], 1.0, a_hi[:].bitcast(F32), MUL, SUB
                    )
                    w_hi = p3_pool.tile([128, T], F32R, tag="w_hi", name="w_hi")
                    nc.vector.tensor_copy(w_hi[:], w4f[:])
                    w_lo = p3_pool.tile([128, T], F32R, tag="w_lo", name="w_lo")
                    nc.vector.scalar_tensor_tensor(
                        w_lo[:], w4f[:], 1.0, w_hi[:].bitcast(F32), MUL, SUB
                    )
                    for h in range(HLOC):
                        r = 32 * h
                        nc.sync.dma_start(k_aug[h][64:65, :], a_hi[r : r + 1, :])
                        nc.sync.dma_start(k_aug[h][65:66, :], a_lo[r : r + 1, :])
                        nc.sync.dma_start(k_aug[h][66:67, :], w_hi[r : r + 1, :])
                        nc.sync.dma_start(k_aug[h][67:68, :], w_lo[r : r + 1, :])

            # ============ P4 + P5: attention & projection ============
            with ExitStack() as p4:
                pair_ps = p4.enter_context(
                    tc.tile_pool(name="pairps", bufs=2, space="PSUM")
                )
                y_ps_pool = p4.enter_context(
                    tc.tile_pool(name="yps", bufs=2, space="PSUM")
                )
                o_ps_pool = p4.enter_context(
                    tc.tile_pool(name="ops", bufs=1, space="PSUM")
                )
                psb_pool = p4.enter_context(tc.tile_pool(name="psb", bufs=3))
                den_pool = p4.enter_context(tc.tile_pool(name="den", bufs=2))
                rep2_pool = p4.enter_context(tc.tile_pool(name="rep2", bufs=2))
                out_pool = p4.enter_context(tc.tile_pool(name="osb", bufs=2))
                dram2 = p4.enter_context(
                    tc.tile_pool(name="dram2", bufs=4, space="DRAM")
                )

                cproj_queue = []
                ynorm_queue = []

                def emit_cproj_unit(tt, cn):
                    del cn
                    ops = o_ps_pool.tile([128, 1024], F32, tag="ops", name="ops")
                    for half in range(2):
                        osl = slice(512 * half, 512 * half + 512)
                        for p in range(2):
                            nc.tensor.matmul(
                                ops[:, osl],
                                y_pack[p][:, 128 * tt : 128 * tt + 128],
                                wproj_sb[p][:, osl],
                                start=(p == 0),
                                stop=(p == 1),
                                skip_group_check=True,
                            )
                    osb = out_pool.tile([128, 1024], BF16, tag="osb", name="osb")
                    nc.vector.tensor_copy(osb[:], ops[:])
                    nc.sync.dma_start(d_out[128 * tt : 128 * tt + 128, :], osb[:])

                for ci in range(NT):
                    isl = slice(NCH * ci, NCH * ci + NCH)
                    for h in range(HLOC):
                        p_pk, s_slot = divmod(h, 2)
                        yps = y_ps_pool.tile([65, NCH], F32, tag="yps", name="yps")
                        nblocks = 4 * ci + 4
                        # groups of 2 blocks: (tj, off) pairs
                        groups = []
                        tj = 0
                        while tj < nblocks:
                            r0, r1 = tj - 4 * ci, tj + 1 - 4 * ci
                            groups.append(
                                (tj, max(0, 128 * r0), tj + 1, max(0, 128 * r1),
                                 r0 >= 0)
                            )
                            tj += 2

                        def emit_scores(g):
                            t0, off0, t1, off1, diag = g
                            pair = pair_ps.tile(
                                [128, 2 * NCH], F32, tag="pair", name="pair"
                            )
                            for half, (tj, off) in enumerate(((t0, off0), (t1, off1))):
                                base = NCH * half
                                if diag:
                                    # preload causal stair (-BIG wedge, 0 below)
                                    nc.tensor.matmul(
                                        pair[:, base + off : base + NCH],
                                        stairT[:],
                                        stairmov[:, 0 : NCH - off],
                                        start=True,
                                        stop=False,
                                        skip_group_check=True,
                                    )
                                nc.tensor.matmul(
                                    pair[:, base + off : base + NCH],
                                    k_aug[h][:, 128 * tj : 128 * tj + 128],
                                    q_aug[h][:, NCH * ci + off : NCH * (ci + 1)],
                                    start=not diag,
                                    stop=True,
                                    skip_group_check=True,
                                )
                            return pair

                        def emit_tail(g, pair, first, last):
                            t0, off0, t1, off1, diag = g
                            psb = psb_pool.tile(
                                [128, 2 * NCH], BF16, tag="psb", name="psb"
                            )
                            if diag:
                                nc.scalar.activation(
                                    psb[:, off0:NCH], pair[:, off0:NCH], AF.Exp
                                )
                                nc.scalar.activation(
                                    psb[:, NCH + off1 : 2 * NCH],
                                    pair[:, NCH + off1 : 2 * NCH],
                                    AF.Exp,
                                )
                            else:
                                nc.scalar.activation(
                                    psb[:], pair[:], AF.Exp
                                )
                            nc.tensor.matmul(
                                yps[:, off0:NCH],
                                v_sb[t0][:, 65 * h : 65 * h + 65],
                                psb[:, off0:NCH],
                                start=first,
                                stop=False,
                                skip_group_check=True,
                            )
                            nc.tensor.matmul(
                                yps[:, off1:NCH],
                                v_sb[t1][:, 65 * h : 65 * h + 65],
                                psb[:, NCH + off1 : 2 * NCH],
                                start=False,
                                stop=last,
                                skip_group_check=True,
                            )

                        prev = None
                        for gi, g in enumerate(groups):
                            pair = emit_scores(g)
                            if prev is not None:
                                pg, ppair, pfirst = prev
                                emit_tail(pg, ppair, pfirst, False)
                                if not pg[4] and cproj_queue:
                                    cproj_queue.pop(0)()
                                elif pg[4] and ynorm_queue:
                                    ynorm_queue.pop(0)()
                            prev = (g, pair, gi == 0)
                        pg, ppair, pfirst = prev
                        emit_tail(pg, ppair, pfirst, True)

                        def make_ynorm(yps, p_pk, s_slot, isl):
                            def fn():
                                # normalize: 1/denom via DVE, broadcast via DRAM
                                den = den_pool.tile([65, NCH], F32, tag="den", name="den")
                                nc.vector.reciprocal(den[64:65, :], yps[64:65, :])
                                drow = dram2.tile([1, NCH], F32, tag="dden", name="dden")
                                nc.sync.dma_start(drow[:], den[64:65, :])
                                rrep = rep2_pool.tile([64, NCH], F32, tag="rrep", name="rrep")
                                nc.sync.dma_start(
                                    rrep[:], drow[0:1, :].partition_broadcast(64)
                                )
                                nc.vector.tensor_tensor(
                                    y_pack[p_pk][64 * s_slot : 64 * s_slot + 64, isl],
                                    yps[0:64, :],
                                    rrep[:],
                                    MUL,
                                )
                            return fn

                        ynorm_queue.append(make_ynorm(yps, p_pk, s_slot, isl))
                        if h == HLOC - 1:
                            while ynorm_queue:
                                ynorm_queue.pop(0)()
                    # queue this chunk's projection; emitted during next chunk
                    for tt in range(4 * ci, 4 * ci + 4):
                        cproj_queue.append(
                            (lambda a: (lambda: emit_cproj_unit(a, 0)))(tt)
                        )
                while cproj_queue:
                    cproj_queue.pop(0)()

    split_excess_waits(nc, max_waits=1)
    return nc


def _host_shards(inputs):
    x = np.asarray(inputs["x"], np.float32)
    Wq = np.asarray(inputs["Wq"], np.float32)
    Wk = np.asarray(inputs["Wk"], np.float32)
    Wv = np.asarray(inputs["Wv"], np.float32)
    Wproj = np.asarray(inputs["Wproj"], np.float32)
    q_rms_w = np.asarray(inputs["q_rms_w"], np.float32)
    k_rms_w = np.asarray(inputs["k_rms_w"], np.float32)
    omega = np.asarray(inputs["omega"], np.float32)
    u = np.asarray(inputs["u"], np.float32)

    import ml_dtypes

    bf16 = ml_dtypes.bfloat16
    slopes = np.asarray(_get_alibi_slopes(H), np.float32)
    omega_eff = np.log1p(np.exp(omega)) * slopes  # softplus(omega) * slopes
    u_n = u / np.maximum(
        np.linalg.norm(u, axis=-1, keepdims=True), U_L2_EPS
    )
    sqrt_d = math.sqrt(D)

    iota = np.arange(T, dtype=np.float32)[None, :]
    ones_row = np.ones((1, T), np.float32)
    ones4 = np.ones((128, HLOC), np.float32).astype(bf16)
    jj = np.arange(128, dtype=np.float32)
    # PSUM-preload stair: -BIG where j>i (k index kk maps to j, col to i)
    stairT = np.where(jj[None, :] > jj[:, None], -BIGF, 0.0).astype(bf16)
    stairmov = np.zeros((128, 512), np.float32)
    stairmov[:, 0:128] = np.eye(128, dtype=np.float32)
    stairmov = stairmov.astype(bf16)
    ssqw_q = np.zeros((128, 97), np.float32)
    ssqw_q[0:64, 0] = 1.0 / D
    ssqw_q[64:128, 32] = 1.0 / D
    ssqw_k = np.zeros((128, 97), np.float32)
    ssqw_k[0:64, 64] = 1.0 / D
    ssqw_k[64:128, 96] = 1.0 / D
    sel_q = np.zeros((97, 128), np.float32)
    sel_q[0, 0:64] = 1.0
    sel_q[32, 64:128] = 1.0
    sel_k = np.zeros((97, 128), np.float32)
    sel_k[64, 0:64] = 1.0
    sel_k[96, 64:128] = 1.0
    qw8 = np.tile(q_rms_w / 8.0, 2)[:, None].astype(np.float32)
    kw = np.tile(k_rms_w, 2)[:, None].astype(np.float32)
    iota97 = np.tile(iota, (128, 1)).astype(np.float32)

    in_maps = []
    for core in range(8):
        b, g = divmod(core, HLOC)
        hs = slice(HLOC * g, HLOC * g + HLOC)
        cs = slice(HD * g, HD * g + HD)
        ucol_ext = np.zeros((D, 512), np.float32)
        for h in range(HLOC):
            ucol_ext[:, 128 * h + 32 * h] = u_n[HLOC * g + h] / sqrt_d
        omg97 = np.zeros((128, 1), np.float32)
        negomg97 = np.zeros((128, 1), np.float32)
        for h in range(HLOC):
            omg97[32 * h, 0] = omega_eff[HLOC * g + h]
            negomg97[32 * h, 0] = -omega_eff[HLOC * g + h]
        in_maps.append(
            {
                "xT": np.ascontiguousarray(x[b].T).astype(bf16),
                "wq": np.ascontiguousarray(Wq[:, cs]).astype(bf16),
                "wk": np.ascontiguousarray(Wk[:, cs]).astype(bf16),
                "wv": np.ascontiguousarray(Wv[:, cs]).astype(bf16),
                "wproj": np.ascontiguousarray(Wproj[cs, :]).astype(bf16),
                "ssqw_q": ssqw_q,
                "ssqw_k": ssqw_k,
                "sel_q": sel_q,
                "sel_k": sel_k,
                "ucol": ucol_ext,
                "omg97": omg97,
                "negomg97": negomg97,
                "iota97": iota97,
                "iota": iota,
                "ones_row": ones_row,
                "ones4": ones4,
                "stairT": stairT,
                "stairmov": stairmov,
                "qw8": qw8,
                "kw": kw,
            }
        )
    return in_maps


def kernel(**inputs):
    from concourse.bass_utils import run_bass_kernel_spmd

    if "nc" not in _cache:
        _cache["nc"] = _build_program()
    nc = _cache["nc"]

    in_maps = _host_shards(inputs)
    res = run_bass_kernel_spmd(nc, in_maps, core_ids=list(range(8)))
    out = np.zeros((B, T, C), np.float32)
    for core in range(8):
        b = core // HLOC
        out[b] += res.results[core]["out"]
    return out
